# revision 10
# baseline (speedup 1.0000x reference)
"""MHCLiteBlock Trainium2 kernel.

Data-parallel over T across 8 NeuronCores (1024 tokens/core); all params
replicated. Per core, per 128-token tile:

  1. Load x [128, 8192] fp32 in 4 chunks; ACT Square+accum -> sum(x^2);
     GPSIMD cast x -> bf16 (raw, unnormalized).
  2. DMA-xbar-transpose x_bf16 -> xT chunks [128c, 128t].
  3. M1 (PE, bf16): projT[32, 128] = W_all @ x^T (RMS norm folded in later).
  4. Small-op chain: transpose proj to [t, 32], scale by inv_rms & alpha, add
     bias, sigmoid/exp, soft-permutation H via perm_aug matmul, normalize.
  5. layer_input (PE): 4 diag(h_pre) matmuls accumulated in PSUM.
  6. M2 (PE, bf16): diff = layer_input @ (W_layer.T - I) + b_layer.
  7. Mixing (PE): out_n = sum_m H[n,m] diag-matmuls over x_m + diag(2*h_post)
     matmul over diff; copy PSUM->SBUF->HBM.

Self-contained: hardcodes shapes; builds the Bass program once and caches it.
"""

import sys

sys.path.insert(0, "/opt/trn_rl_repo")

from contextlib import ExitStack

import ml_dtypes
import numpy as np

import concourse.bass as bass
import concourse.mybir as mybir
import concourse.tile as tile
from concourse import bacc, bass_utils

F32 = mybir.dt.float32
BF16 = mybir.dt.bfloat16
AF = mybir.ActivationFunctionType
ALU = mybir.AluOpType

T, N, C = 8192, 4, 2048
NCF = N * C  # 8192 flattened features
NFACT = 24
NCORES = 8
P = 128  # partitions / tokens per tile
EPS = float(np.finfo(np.float32).eps)


def build_program(t_core: int):
    """Build the per-core Bass program for t_core tokens (t_core % 128 == 0)."""
    nt = t_core // P
    nc = bacc.Bacc(
        "TRN2", target_bir_lowering=False, debug=False, num_devices=NCORES
    )

    x_d = nc.dram_tensor("x", [t_core, NCF], F32, kind="ExternalInput").ap()
    wallt_d = nc.dram_tensor("wallt", [64, P, 32], BF16, kind="ExternalInput").ap()
    wp_d = nc.dram_tensor("wp", [16, P, C], BF16, kind="ExternalInput").ap()
    blayer_d = nc.dram_tensor("blayer", [1, C], BF16, kind="ExternalInput").ap()
    ones_d = nc.dram_tensor("ones1", [1, P], BF16, kind="ExternalInput").ap()
    perm_d = nc.dram_tensor("permaug", [NFACT, 17], F32, kind="ExternalInput").ap()
    ab_d = nc.dram_tensor("alphab", [2, 32], F32, kind="ExternalInput").ap()
    idbf_d = nc.dram_tensor("idbf", [P, P], BF16, kind="ExternalInput").ap()
    idf32_d = nc.dram_tensor("idf32", [P, P], F32, kind="ExternalInput").ap()
    out_d = nc.dram_tensor("out", [t_core, NCF], F32, kind="ExternalOutput").ap()

    with tile.TileContext(nc) as tc:
        _build_body(
            tc, nt, x_d, wallt_d, wp_d, blayer_d, ones_d, perm_d, ab_d,
            idbf_d, idf32_d, out_d,
        )
    nc.compile()
    return nc


def _build_body(
    tc, nt, x_d, wallt_d, wp_d, blayer_d, ones_d, perm_d, ab_d, idbf_d,
    idf32_d, out_d,
):
    nc = tc.nc
    with ExitStack() as ctx:
        singles = ctx.enter_context(tc.tile_pool(name="singles", bufs=1))
        xfp = ctx.enter_context(tc.tile_pool(name="xfp", bufs=3))
        xnp = ctx.enter_context(tc.tile_pool(name="xnp", bufs=2))
        xtp = ctx.enter_context(tc.tile_pool(name="xtp", bufs=4))

        smalls = ctx.enter_context(tc.tile_pool(name="smalls", bufs=2))
        diagp = ctx.enter_context(tc.tile_pool(name="diagp", bufs=2))
        lip = ctx.enter_context(tc.tile_pool(name="lip", bufs=2))
        outp = ctx.enter_context(tc.tile_pool(name="outp", bufs=2))
        ps_small = ctx.enter_context(
            tc.tile_pool(name="ps_small", bufs=2, space="PSUM")
        )
        ps_li = ctx.enter_context(tc.tile_pool(name="ps_li", bufs=2, space="PSUM"))
        ps_diff = ctx.enter_context(
            tc.tile_pool(name="ps_diff", bufs=2, space="PSUM")
        )
        ps_mix = ctx.enter_context(tc.tile_pool(name="ps_mix", bufs=2, space="PSUM"))

        # ---- one-time parameter loads ----
        wp_s = singles.tile([P, 16, C], BF16)
        for k in range(16):
            nc.sync.dma_start(out=wp_s[:, k, :], in_=wp_d[k])
        walls = singles.tile([P, 64, 32], BF16)
        for k in range(64):
            nc.sync.dma_start(out=walls[:, k, :], in_=wallt_d[k])
        blayer_s = singles.tile([1, C], BF16)
        nc.sync.dma_start(out=blayer_s[:], in_=blayer_d[:])
        ones_s = singles.tile([1, P], BF16)
        nc.sync.dma_start(out=ones_s[:], in_=ones_d[:])
        perm_s = singles.tile([NFACT, 17], F32)
        nc.sync.dma_start(out=perm_s[:], in_=perm_d[:])
        idbf_s = singles.tile([P, P], BF16)
        nc.sync.dma_start(out=idbf_s[:], in_=idbf_d[:])
        idf32_s = singles.tile([P, P], F32)
        nc.sync.dma_start(out=idf32_s[:], in_=idf32_d[:])
        # broadcast alpha / bias rows across 128 partitions
        alpha_b = singles.tile([P, 32], F32)
        nc.gpsimd.dma_start(
            out=alpha_b[:],
            in_=bass.AP(tensor=ab_d.tensor, offset=ab_d.offset,
                        ap=[[0, P], [1, 32]]),
        )
        bias_b = singles.tile([P, 32], F32)
        nc.gpsimd.dma_start(
            out=bias_b[:],
            in_=bass.AP(tensor=ab_d.tensor, offset=ab_d.offset + 32,
                        ap=[[0, P], [1, 32]]),
        )
        eps_t = singles.tile([P, 1], F32)
        nc.vector.memset(eps_t[:], EPS)

        for t in range(nt):
            rows = slice(t * P, (t + 1) * P)

            # ---- stage 1: load x, sum(x^2), cast to bf16 ----
            ssqp = smalls.tile([P, N], F32, tag="ssqp")
            xn = xnp.tile([P, NCF], BF16, tag="xn")
            for m in range(N):
                xf = xfp.tile([P, C], F32, tag="xf")
                nc.sync.dma_start(out=xf[:], in_=x_d[rows, m * C:(m + 1) * C])
                nc.gpsimd.tensor_copy(out=xn[:, m * C:(m + 1) * C], in_=xf[:])
                # in-place square: only the free-dim accumulator is consumed
                nc.scalar.activation(
                    out=xf[:], in_=xf[:], func=AF.Square,
                    accum_out=ssqp[:, m:m + 1],
                )

            ssq = smalls.tile([P, 1], F32, tag="ssq")
            nc.vector.tensor_reduce(
                out=ssq[:], in_=ssqp[:], axis=mybir.AxisListType.X, op=ALU.add
            )
            rms = smalls.tile([P, 1], F32, tag="rms")
            nc.scalar.activation(
                out=rms[:], in_=ssq[:], func=AF.Sqrt, bias=eps_t[:],
                scale=1.0 / NCF,
            )
            irms = smalls.tile([P, 1], F32, tag="irms")
            nc.vector.reciprocal(out=irms[:], in_=rms[:])

            # ---- stage 2+3: transpose x (bf16) and M1 projection ----
            projT_p = ps_small.tile([32, P], F32, tag="pssmall")
            for m in range(N):
                xT = xtp.tile([P, 16, P], BF16, tag="xT")
                nc.sync.dma_start_transpose(
                    out=xT[:], in_=xn[:, m * C:(m + 1) * C]
                )
                for kk in range(16):
                    k = m * 16 + kk
                    nc.tensor.matmul(
                        projT_p[:], walls[:, k, :], xT[:, kk, :],
                        start=(k == 0), stop=(k == 63),
                    )

            # ---- stage 4: coefficients ----
            projT_s = smalls.tile([32, P], F32, tag="projT_s")
            nc.scalar.activation(out=projT_s[:], in_=projT_p[:], func=AF.Copy)
            proj_p = ps_small.tile([P, 32], F32, tag="pssmall")
            nc.tensor.transpose(proj_p[:], projT_s[:], idf32_s[0:32, 0:32])
            proj_s = smalls.tile([P, 32], F32, tag="proj_s")
            nc.scalar.activation(out=proj_s[:], in_=proj_p[:], func=AF.Copy)

            # scaled = proj * irms * alpha + bias
            scaled = smalls.tile([P, 32], F32, tag="scaled")
            nc.vector.scalar_tensor_tensor(
                out=scaled[:], in0=proj_s[:], scalar=irms[:], in1=alpha_b[:],
                op0=ALU.mult, op1=ALU.mult,
            )
            nc.vector.tensor_add(scaled[:], scaled[:], bias_b[:])

            hps = smalls.tile([P, 8], F32, tag="hps")
            nc.scalar.activation(out=hps[:], in_=scaled[:, 0:8], func=AF.Sigmoid)
            hp2 = smalls.tile([P, N], F32, tag="hp2")
            nc.vector.tensor_scalar_mul(hp2[:], hps[:, 4:8], 2.0)
            exps = smalls.tile([P, NFACT], F32, tag="exps")
            nc.scalar.activation(out=exps[:], in_=scaled[:, 8:32], func=AF.Exp)

            expsT_p = ps_small.tile([NFACT, P], F32, tag="pssmall")
            nc.tensor.transpose(expsT_p[:], exps[:], idf32_s[:])
            expsT_s = smalls.tile([NFACT, P], F32, tag="expsT_s")
            nc.scalar.activation(out=expsT_s[:], in_=expsT_p[:], func=AF.Copy)

            haug_p = ps_small.tile([P, 17], F32, tag="pssmall")
            nc.tensor.matmul(
                haug_p[:], expsT_s[:], perm_s[:], start=True, stop=True
            )
            hd = smalls.tile([P, 17], F32, tag="hd")
            nc.scalar.activation(out=hd[:], in_=haug_p[:], func=AF.Copy)

            dinv = smalls.tile([P, 1], F32, tag="dinv")
            nc.vector.reciprocal(out=dinv[:], in_=hd[:, 16:17])
            hn = smalls.tile([P, 16], F32, tag="hn")
            nc.vector.tensor_scalar_mul(hn[:], hd[:, 0:16], dinv[:])

            # ---- diag matrices: j=4m+n -> H[n,m]; 16+n -> 2*h_post; 20+m -> h_pre
            diags = diagp.tile([P, 24, P], BF16, tag="diags")
            for j in range(16):
                nc.vector.tensor_scalar_mul(
                    diags[:, j, :], idbf_s[:], hn[:, j:j + 1]
                )
            for n in range(N):
                nc.vector.tensor_scalar_mul(
                    diags[:, 16 + n, :], idbf_s[:], hp2[:, n:n + 1]
                )
            for m in range(N):
                nc.vector.tensor_scalar_mul(
                    diags[:, 20 + m, :], idbf_s[:], hps[:, m:m + 1]
                )

            # ---- stage 5: layer_input = sum_m diag(h_pre_m) @ x_m ----
            libf = lip.tile([P, C], BF16, tag="libf")
            for q in range(4):
                cs = slice(q * 512, (q + 1) * 512)
                li_p = ps_li.tile([P, 512], F32, tag="li")
                for m in range(N):
                    nc.tensor.matmul(
                        li_p[:], diags[:, 20 + m, :],
                        xn[:, m * C + q * 512: m * C + (q + 1) * 512],
                        start=(m == 0), stop=(m == 3),
                    )
                nc.scalar.activation(out=libf[:, cs], in_=li_p[:], func=AF.Copy)

            liT = lip.tile([P, 16, P], BF16, tag="liT")
            nc.sync.dma_start_transpose(out=liT[:], in_=libf[:])

            # ---- stage 6: diff = li @ (W_layer.T - I) + b_layer ----
            diffbf = lip.tile([P, C], BF16, tag="diffbf")
            for q in range(4):
                cs = slice(q * 512, (q + 1) * 512)
                diff_p = ps_diff.tile([P, 512], F32, tag="diff")
                for k in range(16):
                    nc.tensor.matmul(
                        diff_p[:], liT[:, k, :], wp_s[:, k, cs],
                        start=(k == 0), stop=False,
                    )
                nc.tensor.matmul(
                    diff_p[:], ones_s[:], blayer_s[:, cs],
                    start=False, stop=True,
                )
                nc.scalar.activation(out=diffbf[:, cs], in_=diff_p[:], func=AF.Copy)

            # ---- stage 7: mixing ----
            for n in range(N):
                outsb = outp.tile([P, C], F32, tag="outsb")
                for cc in range(4):
                    mix_p = ps_mix.tile([P, 512], F32, tag="mix")
                    for src in range(5):
                        if src < N:
                            j = 4 * src + n
                            rhs = xn[:, src * C + cc * 512: src * C + (cc + 1) * 512]
                        else:
                            j = 16 + n
                            rhs = diffbf[:, cc * 512:(cc + 1) * 512]
                        nc.tensor.matmul(
                            mix_p[:], diags[:, j, :], rhs,
                            start=(src == 0), stop=(src == 4),
                        )
                    nc.scalar.activation(
                        out=outsb[:, cc * 512:(cc + 1) * 512], in_=mix_p[:],
                        func=AF.Copy,
                    )
                nc.sync.dma_start(
                    out=out_d[rows, n * C:(n + 1) * C], in_=outsb[:]
                )


def prep_params(inputs):
    """Host-side parameter preprocessing shared by all cores."""
    bf = ml_dtypes.bfloat16
    W_all = np.asarray(inputs["W_all"], np.float32)
    W_layer = np.asarray(inputs["W_layer"], np.float32)
    b_all = np.asarray(inputs["b_all"], np.float32)
    b_layer = np.asarray(inputs["b_layer"], np.float32)
    perm_mat = np.asarray(inputs["perm_mat"], np.float32)
    a_pre = float(np.asarray(inputs["alpha_pre"]).reshape(-1)[0])
    a_post = float(np.asarray(inputs["alpha_post"]).reshape(-1)[0])
    a_res = float(np.asarray(inputs["alpha_res"]).reshape(-1)[0])

    wallt = np.ascontiguousarray(W_all.T).astype(bf).reshape(64, P, 32)
    wp = (np.ascontiguousarray(W_layer.T) - np.eye(C, dtype=np.float32))
    wp = wp.astype(bf).reshape(16, P, C)
    blayer = b_layer.astype(bf).reshape(1, C)
    ones1 = np.ones((1, P), dtype=bf)
    # perm_aug columns in m-major order: col 4m+n = perm_mat[:, n*4+m]; col 16 = 1
    perm_aug = np.zeros((NFACT, 17), np.float32)
    perm_aug[:, :16] = perm_mat.reshape(NFACT, N, N).transpose(0, 2, 1).reshape(
        NFACT, 16
    )
    perm_aug[:, 16] = 1.0
    alphab = np.zeros((2, 32), np.float32)
    alphab[0, 0:4] = a_pre
    alphab[0, 4:8] = a_post
    alphab[0, 8:32] = a_res
    alphab[1, 0:4] = b_all[0:4]
    alphab[1, 4:8] = b_all[4:8]
    alphab[1, 8:32] = b_all[8:32]
    idbf = np.eye(P, dtype=np.float32).astype(bf)
    idf32 = np.eye(P, dtype=np.float32)
    return {
        "wallt": wallt, "wp": wp, "blayer": blayer, "ones1": ones1,
        "permaug": perm_aug, "alphab": alphab, "idbf": idbf, "idf32": idf32,
    }


_PROGRAM_CACHE = {}


def get_program(t_core):
    if t_core not in _PROGRAM_CACHE:
        _PROGRAM_CACHE[t_core] = build_program(t_core)
    return _PROGRAM_CACHE[t_core]


def run(inputs, trace=False):
    x = np.asarray(inputs["x_streams"], np.float32).reshape(T, NCF)
    params = prep_params(inputs)
    t_core = T // NCORES
    nc = get_program(t_core)
    in_maps = []
    for c in range(NCORES):
        m = dict(params)
        m["x"] = np.ascontiguousarray(x[c * t_core:(c + 1) * t_core])
        in_maps.append(m)
    res = bass_utils.run_bass_kernel_spmd(
        nc, in_maps, core_ids=list(range(NCORES)), trace=trace
    )
    out = np.concatenate([r["out"] for r in res.results], axis=0)
    return out.reshape(T, N, C).astype(np.float32), res


def kernel(**inputs) -> np.ndarray:
    out, _ = run(inputs)
    return out
